# revision 8
# baseline (speedup 1.0000x reference)
"""ArcFace loss on 8 TRN2 NeuronCores (Bass/Tile), class-dim tensor parallel.

loss = -mean_n log(top_n / down_n)
  cos[n,c] = <f_n/|f_n|, w_c/|w_c|>
  top_n    = exp(cos(arccos(ct_n) + A)) with ct_n = cos[n, t_n]
  down_n   = sum_c exp(cos[n,c]) - exp(ct_n) + top_n

Device strategy (per core, C-shard of S=12500 classes):
  - w shard arrives TRANSPOSED [S, 128] so column norms / scaling are
    per-partition ops; PE re-transposes 128-col chunks into matmul layout.
  - main loop: PE matmul (bf16) -> PSUM [128, 1536] -> ScalarE Exp with
    accum_out doing the row-sum for free.  ScalarE is the bottleneck
    (25.6M exps/core at 1 elem/cycle/lane).
  - ct via indirect-DMA row gather from the transposed shard + f32 dot.
  - one AllReduce of [128, 32] partials, epilogue computes
    cos(theta+A) = cA*ct - sA*sqrt(1-ct^2) and the final scalar on-device.
"""

import math
import os
import sys

import numpy as np

for _p in (
    "/root/.axon_site",
    "/root/.axon_site/_ro/trn_rl_repo",
    "/root/.axon_site/_ro/pypackages",
    "/opt/trn_rl_repo",
):
    if os.path.isdir(_p) and _p not in sys.path:
        sys.path.append(_p)

import concourse.bacc as bacc
import concourse.bass as bass
import concourse.tile as tile
from concourse import bass_utils, mybir
from concourse.masks import make_identity

P = 128
N, D, C = 2048, 128, 100000
NCORES = 8
S = C // NCORES              # 12500 classes per core
NM = N // P                  # 16 row tiles
G = 1536                     # PSUM group width (3 banks)
NG = math.ceil(S / G)        # 9 groups (8 x 1536 + 212)
CPG = G // P                 # 12 prep chunks per group
NCH = math.ceil(S / P)       # 98 prep chunks (97 x 128 + 84)
ANGLE = 0.5
F32 = mybir.dt.float32
BF16 = mybir.dt.bfloat16
I32 = mybir.dt.int32
AF = mybir.ActivationFunctionType
ALU = mybir.AluOpType
AX = mybir.AxisListType

TRACE = False
LAST_EXEC_NS = None
LAST_RESULTS = None

_NC_CACHE = None


def _build_body(nc, tc, ctx, feats, wt, tidx, tmask, out):
    cA = float(np.cos(ANGLE))
    sA = float(np.sin(ANGLE))

    const = ctx.enter_context(tc.tile_pool(name="const", bufs=1))
    persist = ctx.enter_context(tc.tile_pool(name="persist", bufs=1))
    work = ctx.enter_context(tc.tile_pool(name="work", bufs=2))
    psA = ctx.enter_context(tc.tile_pool(name="psA", bufs=2, space="PSUM"))
    psB = ctx.enter_context(tc.tile_pool(name="psB", bufs=2, space="PSUM"))
    dram = ctx.enter_context(tc.tile_pool(name="dram", bufs=1, space="DRAM"))

    identity = const.tile([P, P], BF16)
    make_identity(nc, identity)
    ones_col = const.tile([P, 1], F32)
    nc.vector.memset(ones_col, 1.0)

    # persistent SBUF
    w_raw = persist.tile([P, NCH * P], F32, name="w_raw")      # [c_lo, (chunk d)]
    wn = [
        persist.tile([P, min(G, S - g * G)], BF16, name=f"wn{g}")
        for g in range(NG)
    ]                                                          # [d, c] per group
    f_raw = persist.tile([P, NM * P], F32, name="f_raw")       # [n_lo, (m d)]
    f_nat = persist.tile([P, NM * P], F32, name="f_nat")       # normalized
    fT = persist.tile([P, N], BF16, name="fT")                 # [d, n]
    wtg = persist.tile([P, NM * P], F32, name="wtg")           # gathered w cols
    acc = persist.tile([P, NG * NM], F32, name="acc")          # exp row sums
    normsq = persist.tile([P, NCH + NM], F32, name="normsq")
    invall = persist.tile([P, NCH + NM], F32, name="invall")
    ctbuf = persist.tile([P, NM], F32, name="ctbuf")
    ntsq = persist.tile([P, NM], F32, name="ntsq")
    tidx_sb = persist.tile([P, NM], I32, name="tidx_sb")
    tmask_sb = persist.tile([P, NM], F32, name="tmask_sb")
    arbuf = persist.tile([P, 32], F32, name="arbuf")
    arout = persist.tile([P, 32], F32, name="arout")

    nc.vector.memset(normsq, 1.0)  # tail-chunk lanes hold garbage otherwise

    nc.sync.dma_start(tidx_sb[:], tidx)
    nc.sync.dma_start(tmask_sb[:], tmask)

    # gathered target columns: wtg[p, m*P:(m+1)*P] = wt[tidx[p, m], :]
    for m in range(NM):
        nc.gpsimd.indirect_dma_start(
            out=wtg[:, m * P : (m + 1) * P],
            out_offset=None,
            in_=wt,
            in_offset=bass.IndirectOffsetOnAxis(ap=tidx_sb[:, m : m + 1], axis=0),
        )

    # ---- load inputs ----
    nc.sync.dma_start(
        f_raw[:].rearrange("p (m d) -> p m d", d=P),
        feats.rearrange("(m p) d -> p m d", p=P),
    )
    full_rows = (NCH - 1) * P
    nc.sync.dma_start(
        w_raw[:, : full_rows].rearrange("c (j d) -> c j d", d=P),
        wt[:full_rows, :].rearrange("(j c) d -> c j d", c=P),
    )
    tail = S - full_rows  # 84
    nc.sync.dma_start(
        w_raw[:tail, (NCH - 1) * P : NCH * P],
        wt[full_rows:, :],
    )

    # ---- pass 1: squared norms (tensor_tensor_reduce / reciprocal are
    # broken on this runtime; use mul+reduce and exp(-0.5*ln(x)) instead) ----
    for j in range(NCH):
        cw = min(P, S - j * P)
        scr = work.tile([P, P], F32, tag="sqscr")
        nc.vector.tensor_mul(
            scr[:cw],
            w_raw[:cw, j * P : (j + 1) * P],
            w_raw[:cw, j * P : (j + 1) * P],
        )
        nc.vector.tensor_reduce(
            out=normsq[:cw, j : j + 1], in_=scr[:cw], op=ALU.add, axis=AX.X
        )
    for m in range(NM):
        scr = work.tile([P, P], F32, tag="sqscr")
        nc.vector.tensor_mul(
            scr[:], f_raw[:, m * P : (m + 1) * P], f_raw[:, m * P : (m + 1) * P]
        )
        nc.vector.tensor_reduce(
            out=normsq[:, NCH + m : NCH + m + 1], in_=scr[:], op=ALU.add, axis=AX.X
        )
    lnb = work.tile([P, NCH + NM], F32, tag="nrm")
    nc.scalar.activation(lnb[:], normsq[:], AF.Ln)
    nc.scalar.activation(invall[:], lnb[:], AF.Exp, scale=-0.5)

    # ---- pass 2a: features -> f_nat (f32) and fT (bf16, transposed) ----
    for m in range(NM):
        sl = slice(m * P, (m + 1) * P)
        nc.vector.tensor_scalar_mul(
            f_nat[:, sl], f_raw[:, sl], invall[:, NCH + m : NCH + m + 1]
        )
        fb = work.tile([P, P], BF16, tag="fb")
        nc.vector.tensor_copy(fb[:], f_nat[:, sl])
        pt = psA.tile([P, P], BF16, tag="tp")
        nc.tensor.transpose(pt[:], fb[:], identity[:])
        nc.vector.tensor_copy(fT[:, sl], pt[:])

    # ct partials (overlap with main loop): raw dot + gathered-col norms
    for m in range(NM):
        sl = slice(m * P, (m + 1) * P)
        scr = work.tile([P, P], F32, tag="ctscr")
        nc.vector.tensor_mul(scr[:], f_nat[:, sl], wtg[:, sl])
        nc.vector.tensor_reduce(
            out=ctbuf[:, m : m + 1], in_=scr[:], op=ALU.add, axis=AX.X
        )
        scr2 = work.tile([P, P], F32, tag="ctscr")
        nc.vector.tensor_mul(scr2[:], wtg[:, sl], wtg[:, sl])
        nc.vector.tensor_reduce(
            out=ntsq[:, m : m + 1], in_=scr2[:], op=ALU.add, axis=AX.X
        )

    # ---- pass 2b: w chunks -> scaled bf16 -> PE transpose -> wn groups ----
    for g in range(NG):
        gw = wn[g].shape[1]
        for jj in range(math.ceil(gw / P)):
            j = g * CPG + jj
            cw = min(P, S - j * P)
            wb = work.tile([P, P], BF16, tag="wb")
            nc.vector.tensor_scalar_mul(
                wb[:cw], w_raw[:cw, j * P : (j + 1) * P], invall[:cw, j : j + 1]
            )
            pt = psA.tile([P, P], BF16, tag="tp")
            nc.tensor.transpose(pt[:, :cw], wb[:cw, :], identity[:cw, :cw])
            nc.vector.tensor_copy(wn[g][:, jj * P : jj * P + cw], pt[:, :cw])

    # ---- main loop: matmul -> exp(+row-sum) ----
    for g in range(NG):
        c0 = g * G
        cw = wn[g].shape[1]
        for m in range(NM):
            ps = psB.tile([P, G], F32, tag="mm")
            for h in range(math.ceil(cw / 512)):
                h0 = h * 512
                hw = min(512, cw - h0)
                nc.tensor.matmul(
                    ps[:, h0 : h0 + hw],
                    fT[:, m * P : (m + 1) * P],
                    wn[g][:, h0 : h0 + hw],
                    start=True,
                    stop=True,
                )
            scr = work.tile([P, G], BF16, tag="escr")
            nc.scalar.activation(
                scr[:, :cw],
                ps[:, :cw],
                AF.Exp,
                accum_out=acc[:, g * NM + m : g * NM + m + 1],
            )

    # ---- combine partials: arbuf[:, 0:16] = down, [:, 16:32] = masked ct ----
    nc.vector.tensor_reduce(
        out=arbuf[:, 0:NM],
        in_=acc[:].rearrange("p (g m) -> p m g", m=NM),
        op=ALU.add,
        axis=AX.X,
    )
    ntln = work.tile([P, NM], F32, tag="ep")
    nc.scalar.activation(ntln[:], ntsq[:], AF.Ln)
    ntinv = work.tile([P, NM], F32, tag="ep2")
    nc.scalar.activation(ntinv[:], ntln[:], AF.Exp, scale=-0.5)
    ctn = work.tile([P, NM], F32, tag="ep3")
    nc.vector.tensor_mul(ctn[:], ctbuf[:], ntinv[:])
    nc.vector.tensor_mul(arbuf[:, NM : 2 * NM], ctn[:], tmask_sb[:])

    # ---- all-reduce the [128, 32] partials ----
    cc_in = dram.tile([P, 32], F32)
    cc_out = dram.tile([P, 32], F32, addr_space="Shared")
    nc.gpsimd.dma_start(cc_in[:], arbuf[:])
    nc.gpsimd.collective_compute(
        "AllReduce",
        ALU.add,
        replica_groups=[list(range(NCORES))],
        ins=[cc_in[:].opt()],
        outs=[cc_out[:].opt()],
    )
    nc.gpsimd.dma_start(arout[:], cc_out[:])

    # ---- epilogue (identical on every core) ----
    down = arout[:, 0:NM]
    ct = arout[:, NM : 2 * NM]
    e1 = work.tile([P, NM], F32, tag="ep")
    nc.vector.tensor_mul(e1[:], ct, ct)              # ct^2
    sl2 = work.tile([P, NM], F32, tag="ep2")
    nc.scalar.activation(sl2[:], e1[:], AF.Ln, bias=1.0, scale=-1.0)  # ln(1-ct^2)
    st = work.tile([P, NM], F32, tag="ep4")
    nc.scalar.activation(st[:], sl2[:], AF.Exp, scale=0.5)  # sqrt(1-ct^2)
    nc.vector.tensor_scalar_mul(st[:], st[:], -sA)
    ctp = work.tile([P, NM], F32, tag="ep3")
    nc.vector.tensor_scalar_mul(ctp[:], ct, cA)
    nc.vector.tensor_add(ctp[:], ctp[:], st[:])
    ect = work.tile([P, NM], F32, tag="ep")
    nc.scalar.activation(ect[:], ct, AF.Exp)
    top = work.tile([P, NM], F32, tag="ep2")
    nc.scalar.activation(top[:], ctp[:], AF.Exp)
    dp = work.tile([P, NM], F32, tag="ep5")
    nc.vector.tensor_sub(dp[:], down, ect[:])
    nc.vector.tensor_add(dp[:], dp[:], top[:])
    lnv = work.tile([P, NM], F32, tag="ep")
    nc.scalar.activation(lnv[:], dp[:], AF.Ln)
    val = work.tile([P, NM], F32, tag="ep2")
    nc.vector.tensor_sub(val[:], lnv[:], ctp[:])
    row = work.tile([P, 1], F32, tag="ep6")
    nc.vector.tensor_reduce(out=row[:], in_=val[:], op=ALU.add, axis=AX.X)
    tot = psA.tile([1, 1], F32, tag="tp")
    nc.tensor.matmul(tot[:], row[:], ones_col[:], start=True, stop=True)
    res = work.tile([1, 1], F32, tag="ep7")
    nc.vector.tensor_scalar_mul(res[:], tot[:], 1.0 / N)
    nc.sync.dma_start(out, res[:])


_ACT_PATCHED = False


def _patch_act_tables():
    """Make natural_log_exp_and_others the only set offering Exp/Ln so the
    whole kernel uses one ACT table load (no ~2.7us set switches)."""
    global _ACT_PATCHED
    if _ACT_PATCHED:
        return
    _ACT_PATCHED = True
    import concourse.hw_specs as hw_specs

    real = hw_specs.get_activation_tables

    def patched(arch):
        tabs = real(arch)
        out = {}
        for name, funcs in tabs.items():
            if name == "natural_log_exp_and_others":
                out[name] = set(funcs)
            else:
                out[name] = set(funcs) - {AF.Exp, AF.Ln}
        return out

    bacc.get_activation_tables = patched


def _build():
    _patch_act_tables()
    nc = bacc.Bacc(
        "TRN2",
        target_bir_lowering=False,
        debug=False,
        enable_asserts=False,
        num_devices=NCORES,
    )
    feats = nc.dram_tensor("features", [N, D], F32, kind="ExternalInput").ap()
    wt = nc.dram_tensor("wt", [S, D], F32, kind="ExternalInput").ap()
    tidx = nc.dram_tensor("tidx", [P, NM], I32, kind="ExternalInput").ap()
    tmask = nc.dram_tensor("tmask", [P, NM], F32, kind="ExternalInput").ap()
    out = nc.dram_tensor("out", [1, 1], F32, kind="ExternalOutput").ap()
    import contextlib

    with tile.TileContext(nc) as tc:
        with contextlib.ExitStack() as ctx:
            _build_body(nc, tc, ctx, feats, wt, tidx, tmask, out)
    nc.compile()
    return nc


def _get_nc():
    global _NC_CACHE
    if _NC_CACHE is None:
        _NC_CACHE = _build()
    return _NC_CACHE


def kernel(features, target, w):
    global LAST_EXEC_NS, LAST_RESULTS
    features = np.ascontiguousarray(np.asarray(features, dtype=np.float32))
    w = np.asarray(w, dtype=np.float32)
    t = np.asarray(target).astype(np.int64)

    in_maps = []
    for k in range(NCORES):
        wT = np.ascontiguousarray(w[:, k * S : (k + 1) * S].T)
        tl = t - k * S
        own = (tl >= 0) & (tl < S)
        idx = np.where(own, tl, 0).astype(np.int32)
        in_maps.append(
            {
                "features": features,
                "wt": wT,
                "tidx": np.ascontiguousarray(idx.reshape(NM, P).T),
                "tmask": np.ascontiguousarray(
                    own.reshape(NM, P).T.astype(np.float32)
                ),
            }
        )

    nc = _get_nc()
    res = bass_utils.run_bass_kernel_spmd(
        nc, in_maps, core_ids=list(range(NCORES)), trace=TRACE
    )
    LAST_EXEC_NS = res.exec_time_ns
    LAST_RESULTS = res
    val = np.asarray(res.results[0]["out"], dtype=np.float32).reshape(())
    return np.array(val, dtype=np.float32)


if __name__ == "__main__":
    np.random.seed(0)
    f = np.random.randn(N, D).astype(np.float32)
    w = np.random.randn(D, C).astype(np.float32)
    t = np.random.randint(0, C, size=(N,)).astype(np.int64)
    print("loss:", kernel(f, t, w))
